# revision 32
# baseline (speedup 1.0000x reference)
"""Trainium2 Bass kernel for nn_Attention_56530359550323.

Full-input contract: kernel(**inputs) takes the unsharded inputs and returns
the full [4, 2048, 4096] float32 output.

Sharding: 8 cores = 4 batches (data-parallel) x 2 head-groups
(tensor-parallel over the 4 query heads; the single kv head is replicated).
Each core computes a partial output-projection [4096, 2048] (transposed);
the host sums the two partials per batch ("all-reduce after wo") and
transposes back.

Single dense PE stream, all phases overlapped:
  - projections stream x chunks (sync DMA queue) against resident weights
    (scalar DMA queue); the rope+qk-norm epilogue of s-block i runs on
    DVE/ACT underneath s-block i+1's matmuls.
  - 1/x and 1/sqrt(x) are computed as Exp(-Ln(x)) on ACT (both functions
    live in one activation table set), replacing catastrophically serial
    DVE reciprocals.
  - attention per (q-block, head): score chunk matmuls software-pipelined
    two chunks ahead of the PV accumulation; exp on ACT (f16 out), causal
    masks as f16 predicated copies (2x DVE); row-sums batched per group as
    a rank-1 matmul sweep, which gives ACT slack to catch up.
  - output projection per q-block interleaved between attention groups;
    PSUM->SBUF f16 copies alternate DVE/ACT; output DMAs ride the sync
    queue which is idle after phase 1.
"""

import os
import sys
from contextlib import ExitStack

import numpy as np

if "/opt/trn_rl_repo" not in sys.path:
    sys.path.insert(0, "/opt/trn_rl_repo")

import concourse.bass as bass
import concourse.mybir as mybir
import concourse.tile as tile
from concourse import bacc, bass_utils

# ---- problem constants (hardcoded per contract) ----
B, S, D = 4, 2048, 4096
HEAD_DIM = 128
N_HEADS = 4            # local q heads in the reference module
N_KV = 1
ROPE_THETA = 500000.0
EPS = 1e-6
FLOOR_SCALE = 8192.0
ATTN_SCALE = 0.1

P = 128                # partitions
SB = 512               # s-block (q-block) size
NSB = S // SB          # 4
ND = D // P            # 32 contraction chunks for projections
NKCH = S // P          # 16 kv chunks
NCC = D // P           # 32 output column chunks
HG = 2                 # heads per group (tensor-parallel degree 2)

f32 = mybir.dt.float32
f16 = mybir.dt.float16
bf16 = mybir.dt.bfloat16

MM_MODE = os.environ.get("KERNEL_MM_MODE", "f16")

_BUILD_CACHE = {}


def _dt():
    return {"f16": f16, "bf16": bf16}[MM_MODE]


def _np_dt():
    if MM_MODE == "f16":
        return np.float16
    import ml_dtypes

    return ml_dtypes.bfloat16


def build_bass():
    key = MM_MODE
    if key in _BUILD_CACHE:
        return _BUILD_CACHE[key]

    dt = _dt()

    nc = bacc.Bacc("TRN2", target_bir_lowering=False, debug=False)

    # all big tensors arrive pre-tiled host-side so every DMA is a
    # contiguous per-partition read
    xT_d = nc.dram_tensor("xT", (NSB, 8, P, 4, SB), dt, kind="ExternalInput").ap()
    wq_d = nc.dram_tensor("wq_g", (8, P, 4, HG * HEAD_DIM), dt, kind="ExternalInput").ap()
    wk_d = nc.dram_tensor("wk", (8, P, 4, HEAD_DIM), dt, kind="ExternalInput").ap()
    wv_d = nc.dram_tensor("wv", (8, P, 4, HEAD_DIM), dt, kind="ExternalInput").ap()
    wo_d = nc.dram_tensor("wo_g", (P, HG, NCC, P), dt, kind="ExternalInput").ap()
    cs_d = nc.dram_tensor("csT", (P, S), dt, kind="ExternalInput").ap()
    sn_d = nc.dram_tensor("snT", (P, S), dt, kind="ExternalInput").ap()
    qs_d = nc.dram_tensor("qscale", (1, S), f32, kind="ExternalInput").ap()
    out_d = nc.dram_tensor("outT", (NCC, NSB, P, SB), dt, kind="ExternalOutput").ap()
    DEBUG = os.environ.get("KERNEL_DEBUG") == "1"
    if DEBUG:
        dbg_q = nc.dram_tensor("dbg_qT", (P, HG, S), dt, kind="ExternalOutput").ap()
        dbg_k = nc.dram_tensor("dbg_kT", (P, S), dt, kind="ExternalOutput").ap()
        dbg_v = nc.dram_tensor("dbg_v", (P, NKCH, P), dt, kind="ExternalOutput").ap()
        dbg_a = nc.dram_tensor("dbg_aT", (P, HG, S), dt, kind="ExternalOutput").ap()

    # masks for the 4 diagonal chunks of a 512-q block: 1 => future (kill)
    masks_np = np.zeros((P, 4, SB), np.uint16)
    for c in range(4):
        kp = c * P + np.arange(P)[:, None]
        qf = np.arange(SB)[None, :]
        masks_np[:, c, :] = (kp > qf).astype(np.uint16)
    masks_d = nc.inline_tensor(masks_np, name="cmasks")
    ident_d = nc.inline_tensor(np.eye(P, dtype=_np_dt()), name="ident")

    Exp = mybir.ActivationFunctionType.Exp
    Ln = mybir.ActivationFunctionType.Ln
    Square = mybir.ActivationFunctionType.Square

    # Pre-load the one activation table set that covers every ACT function
    # this kernel uses (Exp, Ln, Square, Copy, Identity); otherwise the
    # table-load pass ping-pongs between per-function sets (~1.5us a swap).
    from concourse.hw_specs import get_activation_tables

    _tabs = get_activation_tables(nc.m.arch)
    _need = {Exp, Ln, Square, mybir.ActivationFunctionType.Copy,
             mybir.ActivationFunctionType.Identity}
    _set_id = next(
        i for i, (_, fns) in enumerate(_tabs.items()) if _need <= fns
    )

    with tile.TileContext(nc) as tc, ExitStack() as top:
        nc.scalar.add_instruction(
            mybir.InstLoadActFuncSet(
                name=nc.get_next_instruction_name(),
                ins=[],
                outs=[],
                act_func_set_id=_set_id,
            )
        )
        cpool = top.enter_context(tc.tile_pool(name="consts", bufs=1))
        qkpool = top.enter_context(tc.tile_pool(name="qkv", bufs=1))
        atpool = top.enter_context(tc.tile_pool(name="attn", bufs=1))
        wpool = top.enter_context(tc.tile_pool(name="weights", bufs=1))
        xpool = top.enter_context(tc.tile_pool(name="xstream", bufs=6))
        epool = top.enter_context(tc.tile_pool(name="ep1", bufs=2))
        e2pool = top.enter_context(tc.tile_pool(name="ep2", bufs=2))
        expool = top.enter_context(tc.tile_pool(name="exps", bufs=2))
        opool = top.enter_context(tc.tile_pool(name="ostage", bufs=4))

        # ---- consts (gpsimd DMA queue) ----
        masks_t = cpool.tile([P, 4, SB], mybir.dt.uint16)
        nc.gpsimd.dma_start(masks_t, masks_d.ap())
        ident_t = cpool.tile([P, P], dt)
        nc.gpsimd.dma_start(ident_t, ident_d.ap())
        cs_t = cpool.tile([P, S], dt)
        nc.gpsimd.dma_start(cs_t, cs_d)
        sn_t = cpool.tile([P, S], dt)
        nc.gpsimd.dma_start(sn_t, sn_d)
        qs_t = cpool.tile([1, S], f32)
        nc.gpsimd.dma_start(qs_t, qs_d)
        onescol_t = cpool.tile([P, 1], dt)
        nc.vector.memset(onescol_t, 1.0)
        onesrow_t = cpool.tile([1, P], dt)
        nc.vector.memset(onesrow_t, 1.0)
        zero_t = cpool.tile([P, SB], dt)
        nc.vector.memset(zero_t, 0.0)
        epsb_t = cpool.tile([1, 1], f32)
        nc.vector.memset(epsb_t, float(EPS))

        # ---- weights (scalar/ACT DMA queue, split per d-group) ----
        wq_t = wpool.tile([P, ND, HG * HEAD_DIM], dt)
        wk_t = wpool.tile([P, ND, HEAD_DIM], dt)
        wv_t = wpool.tile([P, ND, HEAD_DIM], dt)
        def load_w_trio(dg):
            dsl = slice(dg * 4, (dg + 1) * 4)
            nc.scalar.dma_start(wq_t[:, dsl, :], wq_d[dg])
            nc.scalar.dma_start(wk_t[:, dsl, :], wk_d[dg])
            nc.scalar.dma_start(wv_t[:, dsl, :], wv_d[dg])

        # only dg0's weights up front; the rest are emitted inside sb0's
        # d-group loop so the scalar DMA queue serves them in need order
        load_w_trio(0)
        wo_t = wpool.tile([P, HG, NCC, P], dt)

        # ---- cross-phase SBUF handoff ----
        qT_t = qkpool.tile([P, HG, S], dt)        # normed+roped+scaled qT
        kT_t = qkpool.tile([P, S], dt)            # normed+roped kT
        vnat_t = qkpool.tile([P, NKCH, P], dt)    # v in natural [s, hd] tiles
        attnT_t = atpool.tile([P, HG, S], dt)

        # =============== phase 1: projections ===============
        with ExitStack() as ph1:
            accps = ph1.enter_context(tc.tile_pool(name="accps", bufs=4, space="PSUM"))
            epps = ph1.enter_context(tc.tile_pool(name="epps", bufs=2, space="PSUM"))
            tpps = ph1.enter_context(tc.tile_pool(name="tpps", bufs=2, space="PSUM"))

            def rope_norm(dst, srcc, sb, with_qscale):
                """srcc: [128, 512] f16 sbuf copy of the projection accumulator
                (feature-major, rotate-half layout). Writes roped+normed
                (+scaled) result into dst. All elementwise work in f16."""
                ss = slice(sb * SB, (sb + 1) * SB)
                rope = epool.tile([P, SB], dt, tag="rope", bufs=3)
                tb = epool.tile([P, SB], dt, tag="tb", bufs=3)
                te, to = srcc[0:64, :], srcc[64:128, :]
                # top half: te*cos - to*sin  (DVE inputs must share base
                # partition, so tmps live in the matching half of tb)
                nc.vector.tensor_mul(rope[0:64, :], te, cs_t[0:64, ss])
                nc.vector.tensor_mul(tb[0:64, :], to, sn_t[64:128, ss])
                nc.vector.tensor_sub(rope[0:64, :], rope[0:64, :], tb[0:64, :])
                # bottom half: to*cos + te*sin
                nc.vector.tensor_mul(rope[64:128, :], to, cs_t[64:128, ss])
                nc.vector.tensor_mul(tb[64:128, :], te, sn_t[0:64, ss])
                nc.vector.tensor_add(
                    rope[64:128, :], rope[64:128, :], tb[64:128, :]
                )

                # sum of squares over features via ones-column matmul
                sq = epool.tile([P, SB], dt, tag="sq", bufs=2)
                nc.scalar.activation(sq, rope, Square)
                ssbc = epps.tile([P, SB], f32, tag="w", name="ssbc")
                nc.tensor.matmul(
                    ssbc[0:1, :], onescol_t[:], sq[:], start=True, stop=True
                )
                # invn = exp(-0.5 * ln(ss/HD + eps)) == rsqrt(mean+eps)
                lnq = epool.tile([1, SB], f32, tag="lnq", bufs=1)
                nc.scalar.activation(
                    lnq, ssbc[0:1, :], Ln, bias=epsb_t[:], scale=1.0 / HEAD_DIM
                )
                recr = epool.tile([1, SB], dt, tag="recr", bufs=2)
                if with_qscale:
                    invn = epool.tile([1, SB], f32, tag="invn", bufs=1)
                    nc.scalar.activation(invn, lnq, Exp, scale=-0.5)
                    # fold in qscale (which carries 1/sqrt(hd) and the
                    # per-position attn temperature)
                    nc.vector.tensor_mul(recr, invn, qs_t[:, ss])
                else:
                    nc.scalar.activation(recr, lnq, Exp, scale=-0.5)
                bc = epps.tile([P, SB], f32, tag="w", name="bc")
                nc.tensor.matmul(bc, onesrow_t[:], recr[:], start=True, stop=True)
                bcs = epool.tile([P, SB], dt, tag="bcs", bufs=2)
                nc.scalar.copy(bcs, bc)
                nc.vector.tensor_mul(dst, rope, bcs)

            def epilogue(sb, copies, kc, v_sb):
                ss = slice(sb * SB, (sb + 1) * SB)
                for h in range(HG):
                    rope_norm(qT_t[:, h, ss], copies[h], sb, with_qscale=True)
                rope_norm(kT_t[:, ss], kc, sb, with_qscale=False)
                for t in range(4):
                    tp_ps = tpps.tile([P, P], dt, tag="tp")
                    nc.tensor.transpose(
                        tp_ps, v_sb[:, t * P:(t + 1) * P], ident_t[:]
                    )
                    nc.vector.tensor_copy(vnat_t[:, sb * 4 + t, :], tp_ps)

            deferred = []
            for sb in range(NSB):
                q_ps = [
                    accps.tile([P, SB], f32, tag="acc", name=f"qps{h}")
                    for h in range(HG)
                ]
                k_ps = accps.tile([P, SB], f32, tag="acc", name="kps")
                v_ps = accps.tile([P, SB], f32, tag="acc", name="vps")
                for dg in range(8):
                    xt = xpool.tile([P, 4, SB], dt, tag="x")
                    nc.sync.dma_start(xt, xT_d[sb, dg])
                    if sb == 0 and dg < 7:
                        load_w_trio(dg + 1)
                    if sb == 0 and dg == 7:
                        nc.scalar.dma_start(wo_t, wo_d)
                    for c in range(4):
                        d = dg * 4 + c
                        st, sp = (d == 0), (d == ND - 1)
                        rhs = xt[:, c, :]
                        for h in range(HG):
                            nc.tensor.matmul(
                                q_ps[h],
                                wq_t[:, d, h * P:(h + 1) * P],
                                rhs,
                                start=st,
                                stop=sp,
                            )
                        nc.tensor.matmul(k_ps, wk_t[:, d, :], rhs, start=st, stop=sp)
                        nc.tensor.matmul(v_ps, wv_t[:, d, :], rhs, start=st, stop=sp)

                # free the accumulator banks fast: PSUM->SBUF f16 copies,
                # q on DVE, k/v on ACT
                copies = []
                for h in range(HG):
                    qc = epool.tile([P, SB], dt, tag=f"qc{h}", bufs=2,
                                    name=f"qcopy{h}")
                    nc.vector.tensor_copy(qc, q_ps[h])
                    copies.append(qc)
                kc = epool.tile([P, SB], dt, tag="kc", bufs=2)
                nc.scalar.copy(kc, k_ps)
                v_sb = epool.tile([P, SB], dt, tag="vc", bufs=2)
                nc.scalar.copy(v_sb, v_ps)
                deferred.append((sb, copies, kc, v_sb))
                # rope/norm/transpose epilogue of the PREVIOUS s-block runs
                # here so its PE ops never stall the in-order PE stream
                if len(deferred) > 1:
                    epilogue(*deferred.pop(0))
            # the last s-block's epilogue can't hide behind another matmul
            # block; its ~2us PE stall is the price of the pool region end
            epilogue(*deferred.pop(0))

        if DEBUG:
            nc.sync.dma_start(dbg_k, kT_t)
            nc.sync.dma_start(dbg_v, vnat_t)

        # =============== phase 2+3: attention + out-projection ===============
        with ExitStack() as ph2:
            scps = ph2.enter_context(tc.tile_pool(name="scps", bufs=4, space="PSUM"))
            pvps = ph2.enter_context(tc.tile_pool(name="pvps", bufs=2, space="PSUM"))
            rsps = ph2.enter_context(tc.tile_pool(name="rsps", bufs=2, space="PSUM"))

            def finalize(h, qs_sl, pv_ps, rs_ps):
                # release pv bank first, then 1/rowsum = exp(-ln(rs)) on ACT
                pvs = e2pool.tile([P, SB], dt, tag="pvs")
                nc.vector.tensor_copy(pvs, pv_ps)
                lnr = e2pool.tile([1, SB], f32, tag="lnr", bufs=1)
                nc.scalar.activation(lnr, rs_ps, Ln)
                recr = e2pool.tile([1, SB], dt, tag="rec2")
                nc.scalar.activation(recr, lnr, Exp, scale=-1.0)
                bc_ps = scps.tile([P, SB], f32, tag="sc", name="obc")
                nc.tensor.matmul(bc_ps, onesrow_t[:], recr[:], start=True, stop=True)
                bc_sb = e2pool.tile([P, SB], dt, tag="bcc")
                nc.scalar.copy(bc_sb, bc_ps)
                nc.vector.tensor_mul(attnT_t[:, h, qs_sl], pvs, bc_sb)

            def outproj(qb):
                qsl = slice(qb * SB, (qb + 1) * SB)
                for cc in range(NCC):
                    o_ps = scps.tile([P, SB], f32, tag="sc", name="ops")
                    for h in range(HG):
                        nc.tensor.matmul(
                            o_ps,
                            wo_t[:, h, cc, :],
                            attnT_t[:, h, qsl],
                            start=(h == 0),
                            stop=(h == HG - 1),
                        )
                    o_sb = opool.tile([P, SB], dt, tag="oc", bufs=6)
                    if cc % 2 == 0:
                        nc.vector.tensor_copy(o_sb, o_ps)
                    else:
                        nc.scalar.copy(o_sb, o_ps)
                    nc.sync.dma_start(out_d[cc, qb], o_sb)

            pending = []
            for qb in range(NSB):
                qs_sl = slice(qb * SB, (qb + 1) * SB)
                nch = 4 * qb + 4
                for h in range(HG):
                    pv_ps = pvps.tile([P, SB], f32, tag="pv")
                    e_t = expool.tile([P, NKCH, SB], dt, tag="e")
                    qt = qT_t[:, h, qs_sl]
                    # chunk sweep, pv lagging sc/exp by 2 chunks
                    for c in range(nch):
                        sc_ps = scps.tile([P, SB], f32, tag="sc", name="scps")
                        nc.tensor.matmul(
                            sc_ps,
                            kT_t[:, c * P:(c + 1) * P],
                            qt,
                            start=True,
                            stop=True,
                        )
                        nc.scalar.activation(e_t[:, c, :], sc_ps, Exp)
                        if c >= 4 * qb:
                            nc.vector.copy_predicated(
                                e_t[:, c, :], masks_t[:, c - 4 * qb, :], zero_t
                            )
                        if c >= 2:
                            cl = c - 2
                            nc.tensor.matmul(
                                pv_ps, vnat_t[:, cl, :], e_t[:, cl, :],
                                start=(cl == 0), stop=False,
                            )
                    for cl in (nch - 2, nch - 1):
                        nc.tensor.matmul(
                            pv_ps, vnat_t[:, cl, :], e_t[:, cl, :],
                            start=(cl == 0), stop=(cl == nch - 1),
                        )
                    if pending:
                        fqb, fh, *rest = pending.pop(0)
                        finalize(fh, *rest)
                        if fh == HG - 1:
                            outproj(fqb)
                    # batched row-sum sweep (pure PE, lets ACT catch up)
                    rs_ps = rsps.tile([P, SB], f32, tag="rs", name="rsps")
                    for c in range(nch):
                        nc.tensor.matmul(
                            rs_ps[0:1, :], onescol_t[:], e_t[:, c, :],
                            start=(c == 0), stop=(c == nch - 1),
                        )
                    pending.append((qb, h, qs_sl, pv_ps, rs_ps[0:1, :]))
            for fqb, fh, *rest in pending:
                finalize(fh, *rest)
                if fh == HG - 1:
                    outproj(fqb)

        if DEBUG:
            nc.sync.dma_start(dbg_q, qT_t)
            nc.sync.dma_start(dbg_a, attnT_t)

    nc.compile()
    _BUILD_CACHE[key] = nc
    return nc


def _host_prep(x, positions, wq, wk, wv, wo):
    """Returns per-core input maps."""
    npdt = _np_dt()

    pos_f = positions.astype(np.float32)
    inv_freq = (
        1.0
        / (ROPE_THETA ** (np.arange(0, HEAD_DIM, 2, dtype=np.float32) / HEAD_DIM))
    ).astype(np.float32)
    ang = pos_f[:, None] * inv_freq[None, :]        # [S, 64]
    csT_h = np.cos(ang).T.astype(npdt)              # [64, S]
    snT_h = np.sin(ang).T.astype(npdt)
    # duplicated halves so rope operands share base partitions
    csT = np.ascontiguousarray(np.concatenate([csT_h, csT_h], axis=0))  # [128, S]
    snT = np.ascontiguousarray(np.concatenate([snT_h, snT_h], axis=0))
    attn_scales = (
        np.log(np.floor((pos_f + 1.0) / FLOOR_SCALE) + 1.0) * ATTN_SCALE + 1.0
    )
    qscale = (attn_scales / np.sqrt(np.float32(HEAD_DIM))).astype(np.float32)[None, :]

    # rotate-half permutation of q/k feature dims (per head), folded into
    # the projection weight columns: permuted feature j<64 <- 2j, j>=64 <- 2(j-64)+1
    perm = np.concatenate([np.arange(0, HEAD_DIM, 2), np.arange(1, HEAD_DIM, 2)])
    wq_p = wq.reshape(D, N_HEADS, HEAD_DIM)[:, :, perm].reshape(D, N_HEADS * HEAD_DIM)
    wk_p = wk[:, perm]

    def tile_x(xT):
        # [D, S] -> [sb, dg, p, c, s]
        return np.ascontiguousarray(
            xT.reshape(8, 4, P, NSB, SB).transpose(3, 0, 2, 1, 4)
        )

    def tile_w(w):
        # [D, m] -> [dg, p, c, m]
        m = w.shape[1]
        return np.ascontiguousarray(
            w.reshape(8, 4, P, m).transpose(0, 2, 1, 3)
        )

    def tile_wo(wg):
        # [256, D] -> [p, hh, cc, q]
        return np.ascontiguousarray(
            wg.reshape(HG, P, NCC, P).transpose(1, 0, 2, 3)
        )

    in_maps = []
    for core in range(8):
        b, g = core // 2, core % 2
        xT = np.ascontiguousarray(x[b].T).astype(npdt, copy=False)
        in_maps.append(
            {
                "xT": tile_x(xT),
                "wq_g": tile_w(
                    wq_p[:, g * HG * HEAD_DIM:(g + 1) * HG * HEAD_DIM].astype(npdt)
                ),
                "wk": tile_w(wk_p.astype(npdt)),
                "wv": tile_w(wv.astype(npdt)),
                "wo_g": tile_wo(
                    wo[g * HG * HEAD_DIM:(g + 1) * HG * HEAD_DIM, :].astype(npdt)
                ),
                "csT": csT,
                "snT": snT,
                "qscale": qscale,
            }
        )
    return in_maps


def kernel(x, positions, wq, wk, wv, wo, _trace=False, _trace_kwargs=None):
    x = np.asarray(x, np.float32)
    positions = np.asarray(positions)
    wq = np.asarray(wq, np.float32)
    wk = np.asarray(wk, np.float32)
    wv = np.asarray(wv, np.float32)
    wo = np.asarray(wo, np.float32)

    nc = build_bass()
    in_maps = _host_prep(x, positions, wq, wk, wv, wo)
    res = bass_utils.run_bass_kernel_spmd(
        nc, in_maps, core_ids=list(range(8)), trace=_trace,
        **(_trace_kwargs or {}),
    )
    kernel.last_results = res

    out = np.empty((B, S, D), np.float32)
    for b in range(B):
        pa = res.results[2 * b]["outT"].astype(np.float32)
        pb = res.results[2 * b + 1]["outT"].astype(np.float32)
        full = (pa + pb).transpose(0, 2, 1, 3).reshape(D, S)
        out[b] = full.T
    return out


# revision 33
# speedup vs baseline: 1.0030x; 1.0030x over previous
"""Trainium2 Bass kernel for nn_Attention_56530359550323.

Full-input contract: kernel(**inputs) takes the unsharded inputs and returns
the full [4, 2048, 4096] float32 output.

Sharding: 8 cores = 4 batches (data-parallel) x 2 head-groups
(tensor-parallel over the 4 query heads; the single kv head is replicated).
Each core computes a partial output-projection [4096, 2048] (transposed);
the host sums the two partials per batch ("all-reduce after wo") and
transposes back.

Single dense PE stream, all phases overlapped:
  - projections stream x chunks (sync DMA queue) against resident weights
    (scalar DMA queue); the rope+qk-norm epilogue of s-block i runs on
    DVE/ACT underneath s-block i+1's matmuls.
  - 1/x and 1/sqrt(x) are computed as Exp(-Ln(x)) on ACT (both functions
    live in one activation table set), replacing catastrophically serial
    DVE reciprocals.
  - attention per (q-block, head): score chunk matmuls software-pipelined
    two chunks ahead of the PV accumulation; exp on ACT (f16 out), causal
    masks as f16 predicated copies (2x DVE); row-sums batched per group as
    a rank-1 matmul sweep, which gives ACT slack to catch up.
  - output projection per q-block interleaved between attention groups;
    PSUM->SBUF f16 copies alternate DVE/ACT; output DMAs ride the sync
    queue which is idle after phase 1.
"""

import os
import sys
from contextlib import ExitStack

import numpy as np

if "/opt/trn_rl_repo" not in sys.path:
    sys.path.insert(0, "/opt/trn_rl_repo")

import concourse.bass as bass
import concourse.mybir as mybir
import concourse.tile as tile
from concourse import bacc, bass_utils

# ---- problem constants (hardcoded per contract) ----
B, S, D = 4, 2048, 4096
HEAD_DIM = 128
N_HEADS = 4            # local q heads in the reference module
N_KV = 1
ROPE_THETA = 500000.0
EPS = 1e-6
FLOOR_SCALE = 8192.0
ATTN_SCALE = 0.1

P = 128                # partitions
SB = 512               # s-block (q-block) size
NSB = S // SB          # 4
ND = D // P            # 32 contraction chunks for projections
NKCH = S // P          # 16 kv chunks
NCC = D // P           # 32 output column chunks
HG = 2                 # heads per group (tensor-parallel degree 2)

f32 = mybir.dt.float32
f16 = mybir.dt.float16
bf16 = mybir.dt.bfloat16

MM_MODE = os.environ.get("KERNEL_MM_MODE", "f16")

_BUILD_CACHE = {}


def _dt():
    return {"f16": f16, "bf16": bf16}[MM_MODE]


def _np_dt():
    if MM_MODE == "f16":
        return np.float16
    import ml_dtypes

    return ml_dtypes.bfloat16


def build_bass():
    key = MM_MODE
    if key in _BUILD_CACHE:
        return _BUILD_CACHE[key]

    dt = _dt()

    nc = bacc.Bacc("TRN2", target_bir_lowering=False, debug=False)

    # all big tensors arrive pre-tiled host-side so every DMA is a
    # contiguous per-partition read
    xT_d = nc.dram_tensor("xT", (NSB, 8, P, 4, SB), dt, kind="ExternalInput").ap()
    wq_d = nc.dram_tensor("wq_g", (8, P, 4, HG * HEAD_DIM), dt, kind="ExternalInput").ap()
    wk_d = nc.dram_tensor("wk", (8, P, 4, HEAD_DIM), dt, kind="ExternalInput").ap()
    wv_d = nc.dram_tensor("wv", (8, P, 4, HEAD_DIM), dt, kind="ExternalInput").ap()
    wo_d = nc.dram_tensor("wo_g", (P, HG, NCC, P), dt, kind="ExternalInput").ap()
    cs_d = nc.dram_tensor("csT", (P, S), dt, kind="ExternalInput").ap()
    sn_d = nc.dram_tensor("snT", (P, S), dt, kind="ExternalInput").ap()
    qs_d = nc.dram_tensor("qscale", (1, S), f32, kind="ExternalInput").ap()
    out_d = nc.dram_tensor("outT", (NCC, NSB, P, SB), dt, kind="ExternalOutput").ap()
    DEBUG = os.environ.get("KERNEL_DEBUG") == "1"
    if DEBUG:
        dbg_q = nc.dram_tensor("dbg_qT", (P, HG, S), dt, kind="ExternalOutput").ap()
        dbg_k = nc.dram_tensor("dbg_kT", (P, S), dt, kind="ExternalOutput").ap()
        dbg_v = nc.dram_tensor("dbg_v", (P, NKCH, P), dt, kind="ExternalOutput").ap()
        dbg_a = nc.dram_tensor("dbg_aT", (P, HG, S), dt, kind="ExternalOutput").ap()

    # masks for the 4 diagonal chunks of a 512-q block: 1 => future (kill)
    masks_np = np.zeros((P, 4, SB), np.uint16)
    for c in range(4):
        kp = c * P + np.arange(P)[:, None]
        qf = np.arange(SB)[None, :]
        masks_np[:, c, :] = (kp > qf).astype(np.uint16)
    masks_d = nc.inline_tensor(masks_np, name="cmasks")
    ident_d = nc.inline_tensor(np.eye(P, dtype=_np_dt()), name="ident")

    Exp = mybir.ActivationFunctionType.Exp
    Ln = mybir.ActivationFunctionType.Ln
    Square = mybir.ActivationFunctionType.Square

    # Pre-load the one activation table set that covers every ACT function
    # this kernel uses (Exp, Ln, Square, Copy, Identity); otherwise the
    # table-load pass ping-pongs between per-function sets (~1.5us a swap).
    from concourse.hw_specs import get_activation_tables

    _tabs = get_activation_tables(nc.m.arch)
    _need = {Exp, Ln, Square, mybir.ActivationFunctionType.Copy,
             mybir.ActivationFunctionType.Identity}
    _set_id = next(
        i for i, (_, fns) in enumerate(_tabs.items()) if _need <= fns
    )

    with tile.TileContext(nc) as tc, ExitStack() as top:
        nc.scalar.add_instruction(
            mybir.InstLoadActFuncSet(
                name=nc.get_next_instruction_name(),
                ins=[],
                outs=[],
                act_func_set_id=_set_id,
            )
        )
        cpool = top.enter_context(tc.tile_pool(name="consts", bufs=1))
        qkpool = top.enter_context(tc.tile_pool(name="qkv", bufs=1))
        atpool = top.enter_context(tc.tile_pool(name="attn", bufs=1))
        wpool = top.enter_context(tc.tile_pool(name="weights", bufs=1))
        xpool = top.enter_context(tc.tile_pool(name="xstream", bufs=7))
        epool = top.enter_context(tc.tile_pool(name="ep1", bufs=2))
        e2pool = top.enter_context(tc.tile_pool(name="ep2", bufs=2))
        expool = top.enter_context(tc.tile_pool(name="exps", bufs=2))
        opool = top.enter_context(tc.tile_pool(name="ostage", bufs=4))

        # ---- consts (gpsimd DMA queue) ----
        masks_t = cpool.tile([P, 4, SB], mybir.dt.uint16)
        nc.gpsimd.dma_start(masks_t, masks_d.ap())
        ident_t = cpool.tile([P, P], dt)
        nc.gpsimd.dma_start(ident_t, ident_d.ap())
        cs_t = cpool.tile([P, S], dt)
        nc.gpsimd.dma_start(cs_t, cs_d)
        sn_t = cpool.tile([P, S], dt)
        nc.gpsimd.dma_start(sn_t, sn_d)
        qs_t = cpool.tile([1, S], f32)
        nc.gpsimd.dma_start(qs_t, qs_d)
        onescol_t = cpool.tile([P, 1], dt)
        nc.vector.memset(onescol_t, 1.0)
        onesrow_t = cpool.tile([1, P], dt)
        nc.vector.memset(onesrow_t, 1.0)
        zero_t = cpool.tile([P, SB], dt)
        nc.vector.memset(zero_t, 0.0)
        epsb_t = cpool.tile([1, 1], f32)
        nc.vector.memset(epsb_t, float(EPS))

        # ---- weights (scalar/ACT DMA queue, split per d-group) ----
        wq_t = wpool.tile([P, ND, HG * HEAD_DIM], dt)
        wk_t = wpool.tile([P, ND, HEAD_DIM], dt)
        wv_t = wpool.tile([P, ND, HEAD_DIM], dt)
        def load_w_trio(dg):
            dsl = slice(dg * 4, (dg + 1) * 4)
            nc.scalar.dma_start(wq_t[:, dsl, :], wq_d[dg])
            nc.scalar.dma_start(wk_t[:, dsl, :], wk_d[dg])
            nc.scalar.dma_start(wv_t[:, dsl, :], wv_d[dg])

        # only dg0's weights up front; the rest are emitted inside sb0's
        # d-group loop so the scalar DMA queue serves them in need order
        load_w_trio(0)
        wo_t = wpool.tile([P, HG, NCC, P], dt)

        # ---- cross-phase SBUF handoff ----
        qT_t = qkpool.tile([P, HG, S], dt)        # normed+roped+scaled qT
        kT_t = qkpool.tile([P, S], dt)            # normed+roped kT
        vnat_t = qkpool.tile([P, NKCH, P], dt)    # v in natural [s, hd] tiles
        attnT_t = atpool.tile([P, HG, S], dt)

        # =============== phase 1: projections ===============
        with ExitStack() as ph1:
            accps = ph1.enter_context(tc.tile_pool(name="accps", bufs=4, space="PSUM"))
            epps = ph1.enter_context(tc.tile_pool(name="epps", bufs=2, space="PSUM"))
            tpps = ph1.enter_context(tc.tile_pool(name="tpps", bufs=2, space="PSUM"))

            def rope_norm(dst, srcc, sb, with_qscale):
                """srcc: [128, 512] f16 sbuf copy of the projection accumulator
                (feature-major, rotate-half layout). Writes roped+normed
                (+scaled) result into dst. All elementwise work in f16."""
                ss = slice(sb * SB, (sb + 1) * SB)
                rope = epool.tile([P, SB], dt, tag="rope", bufs=3)
                tb = epool.tile([P, SB], dt, tag="tb", bufs=3)
                te, to = srcc[0:64, :], srcc[64:128, :]
                # top half: te*cos - to*sin  (DVE inputs must share base
                # partition, so tmps live in the matching half of tb)
                nc.vector.tensor_mul(rope[0:64, :], te, cs_t[0:64, ss])
                nc.vector.tensor_mul(tb[0:64, :], to, sn_t[64:128, ss])
                nc.vector.tensor_sub(rope[0:64, :], rope[0:64, :], tb[0:64, :])
                # bottom half: to*cos + te*sin
                nc.vector.tensor_mul(rope[64:128, :], to, cs_t[64:128, ss])
                nc.vector.tensor_mul(tb[64:128, :], te, sn_t[0:64, ss])
                nc.vector.tensor_add(
                    rope[64:128, :], rope[64:128, :], tb[64:128, :]
                )

                # sum of squares over features via ones-column matmul
                sq = epool.tile([P, SB], dt, tag="sq", bufs=2)
                nc.scalar.activation(sq, rope, Square)
                ssbc = epps.tile([P, SB], f32, tag="w", name="ssbc")
                nc.tensor.matmul(
                    ssbc[0:1, :], onescol_t[:], sq[:], start=True, stop=True
                )
                # invn = exp(-0.5 * ln(ss/HD + eps)) == rsqrt(mean+eps)
                lnq = epool.tile([1, SB], f32, tag="lnq", bufs=1)
                nc.scalar.activation(
                    lnq, ssbc[0:1, :], Ln, bias=epsb_t[:], scale=1.0 / HEAD_DIM
                )
                recr = epool.tile([1, SB], dt, tag="recr", bufs=2)
                if with_qscale:
                    invn = epool.tile([1, SB], f32, tag="invn", bufs=1)
                    nc.scalar.activation(invn, lnq, Exp, scale=-0.5)
                    # fold in qscale (which carries 1/sqrt(hd) and the
                    # per-position attn temperature)
                    nc.vector.tensor_mul(recr, invn, qs_t[:, ss])
                else:
                    nc.scalar.activation(recr, lnq, Exp, scale=-0.5)
                bc = epps.tile([P, SB], f32, tag="w", name="bc")
                nc.tensor.matmul(bc, onesrow_t[:], recr[:], start=True, stop=True)
                bcs = epool.tile([P, SB], dt, tag="bcs", bufs=2)
                nc.scalar.copy(bcs, bc)
                nc.vector.tensor_mul(dst, rope, bcs)

            def epilogue(sb, copies, kc, v_sb):
                ss = slice(sb * SB, (sb + 1) * SB)
                for h in range(HG):
                    rope_norm(qT_t[:, h, ss], copies[h], sb, with_qscale=True)
                rope_norm(kT_t[:, ss], kc, sb, with_qscale=False)
                for t in range(4):
                    tp_ps = tpps.tile([P, P], dt, tag="tp")
                    nc.tensor.transpose(
                        tp_ps, v_sb[:, t * P:(t + 1) * P], ident_t[:]
                    )
                    nc.vector.tensor_copy(vnat_t[:, sb * 4 + t, :], tp_ps)

            deferred = []
            for sb in range(NSB):
                q_ps = [
                    accps.tile([P, SB], f32, tag="acc", name=f"qps{h}")
                    for h in range(HG)
                ]
                k_ps = accps.tile([P, SB], f32, tag="acc", name="kps")
                v_ps = accps.tile([P, SB], f32, tag="acc", name="vps")
                for dg in range(8):
                    xt = xpool.tile([P, 4, SB], dt, tag="x")
                    nc.sync.dma_start(xt, xT_d[sb, dg])
                    if sb == 0 and dg < 7:
                        load_w_trio(dg + 1)
                    # wo is first needed ~175us in; keep its 2MB out of
                    # sb0's DMA-saturated window
                    if sb == 1 and dg == 0:
                        nc.scalar.dma_start(wo_t, wo_d)
                    for c in range(4):
                        d = dg * 4 + c
                        st, sp = (d == 0), (d == ND - 1)
                        rhs = xt[:, c, :]
                        for h in range(HG):
                            nc.tensor.matmul(
                                q_ps[h],
                                wq_t[:, d, h * P:(h + 1) * P],
                                rhs,
                                start=st,
                                stop=sp,
                            )
                        nc.tensor.matmul(k_ps, wk_t[:, d, :], rhs, start=st, stop=sp)
                        nc.tensor.matmul(v_ps, wv_t[:, d, :], rhs, start=st, stop=sp)

                # free the accumulator banks fast: PSUM->SBUF f16 copies,
                # q on DVE, k/v on ACT
                copies = []
                for h in range(HG):
                    qc = epool.tile([P, SB], dt, tag=f"qc{h}", bufs=2,
                                    name=f"qcopy{h}")
                    nc.vector.tensor_copy(qc, q_ps[h])
                    copies.append(qc)
                kc = epool.tile([P, SB], dt, tag="kc", bufs=2)
                nc.scalar.copy(kc, k_ps)
                v_sb = epool.tile([P, SB], dt, tag="vc", bufs=2)
                nc.scalar.copy(v_sb, v_ps)
                deferred.append((sb, copies, kc, v_sb))
                # rope/norm/transpose epilogue of the PREVIOUS s-block runs
                # here so its PE ops never stall the in-order PE stream
                if len(deferred) > 1:
                    epilogue(*deferred.pop(0))
            # the last s-block's epilogue can't hide behind another matmul
            # block; its ~2us PE stall is the price of the pool region end
            epilogue(*deferred.pop(0))

        if DEBUG:
            nc.sync.dma_start(dbg_k, kT_t)
            nc.sync.dma_start(dbg_v, vnat_t)

        # =============== phase 2+3: attention + out-projection ===============
        with ExitStack() as ph2:
            scps = ph2.enter_context(tc.tile_pool(name="scps", bufs=4, space="PSUM"))
            pvps = ph2.enter_context(tc.tile_pool(name="pvps", bufs=2, space="PSUM"))
            rsps = ph2.enter_context(tc.tile_pool(name="rsps", bufs=2, space="PSUM"))

            def finalize(h, qs_sl, pv_ps, rs_ps):
                # release pv bank first, then 1/rowsum = exp(-ln(rs)) on ACT
                pvs = e2pool.tile([P, SB], dt, tag="pvs")
                nc.vector.tensor_copy(pvs, pv_ps)
                lnr = e2pool.tile([1, SB], f32, tag="lnr", bufs=1)
                nc.scalar.activation(lnr, rs_ps, Ln)
                recr = e2pool.tile([1, SB], dt, tag="rec2")
                nc.scalar.activation(recr, lnr, Exp, scale=-1.0)
                bc_ps = scps.tile([P, SB], f32, tag="sc", name="obc")
                nc.tensor.matmul(bc_ps, onesrow_t[:], recr[:], start=True, stop=True)
                bc_sb = e2pool.tile([P, SB], dt, tag="bcc")
                nc.scalar.copy(bc_sb, bc_ps)
                nc.vector.tensor_mul(attnT_t[:, h, qs_sl], pvs, bc_sb)

            def outproj(qb):
                qsl = slice(qb * SB, (qb + 1) * SB)
                for cc in range(NCC):
                    o_ps = scps.tile([P, SB], f32, tag="sc", name="ops")
                    for h in range(HG):
                        nc.tensor.matmul(
                            o_ps,
                            wo_t[:, h, cc, :],
                            attnT_t[:, h, qsl],
                            start=(h == 0),
                            stop=(h == HG - 1),
                        )
                    o_sb = opool.tile([P, SB], dt, tag="oc", bufs=6)
                    if cc % 2 == 0:
                        nc.vector.tensor_copy(o_sb, o_ps)
                    else:
                        nc.scalar.copy(o_sb, o_ps)
                    nc.sync.dma_start(out_d[cc, qb], o_sb)

            pending = []
            for qb in range(NSB):
                qs_sl = slice(qb * SB, (qb + 1) * SB)
                nch = 4 * qb + 4
                for h in range(HG):
                    pv_ps = pvps.tile([P, SB], f32, tag="pv")
                    e_t = expool.tile([P, NKCH, SB], dt, tag="e")
                    qt = qT_t[:, h, qs_sl]
                    # chunk sweep, pv lagging sc/exp by 2 chunks
                    for c in range(nch):
                        sc_ps = scps.tile([P, SB], f32, tag="sc", name="scps")
                        nc.tensor.matmul(
                            sc_ps,
                            kT_t[:, c * P:(c + 1) * P],
                            qt,
                            start=True,
                            stop=True,
                        )
                        nc.scalar.activation(e_t[:, c, :], sc_ps, Exp)
                        if c >= 4 * qb:
                            nc.vector.copy_predicated(
                                e_t[:, c, :], masks_t[:, c - 4 * qb, :], zero_t
                            )
                        if c >= 2:
                            cl = c - 2
                            nc.tensor.matmul(
                                pv_ps, vnat_t[:, cl, :], e_t[:, cl, :],
                                start=(cl == 0), stop=False,
                            )
                    for cl in (nch - 2, nch - 1):
                        nc.tensor.matmul(
                            pv_ps, vnat_t[:, cl, :], e_t[:, cl, :],
                            start=(cl == 0), stop=(cl == nch - 1),
                        )
                    if pending:
                        fqb, fh, *rest = pending.pop(0)
                        finalize(fh, *rest)
                        if fh == HG - 1:
                            outproj(fqb)
                    # batched row-sum sweep (pure PE, lets ACT catch up)
                    rs_ps = rsps.tile([P, SB], f32, tag="rs", name="rsps")
                    for c in range(nch):
                        nc.tensor.matmul(
                            rs_ps[0:1, :], onescol_t[:], e_t[:, c, :],
                            start=(c == 0), stop=(c == nch - 1),
                        )
                    pending.append((qb, h, qs_sl, pv_ps, rs_ps[0:1, :]))
            for fqb, fh, *rest in pending:
                finalize(fh, *rest)
                if fh == HG - 1:
                    outproj(fqb)

        if DEBUG:
            nc.sync.dma_start(dbg_q, qT_t)
            nc.sync.dma_start(dbg_a, attnT_t)

    nc.compile()
    _BUILD_CACHE[key] = nc
    return nc


def _host_prep(x, positions, wq, wk, wv, wo):
    """Returns per-core input maps."""
    npdt = _np_dt()

    pos_f = positions.astype(np.float32)
    inv_freq = (
        1.0
        / (ROPE_THETA ** (np.arange(0, HEAD_DIM, 2, dtype=np.float32) / HEAD_DIM))
    ).astype(np.float32)
    ang = pos_f[:, None] * inv_freq[None, :]        # [S, 64]
    csT_h = np.cos(ang).T.astype(npdt)              # [64, S]
    snT_h = np.sin(ang).T.astype(npdt)
    # duplicated halves so rope operands share base partitions
    csT = np.ascontiguousarray(np.concatenate([csT_h, csT_h], axis=0))  # [128, S]
    snT = np.ascontiguousarray(np.concatenate([snT_h, snT_h], axis=0))
    attn_scales = (
        np.log(np.floor((pos_f + 1.0) / FLOOR_SCALE) + 1.0) * ATTN_SCALE + 1.0
    )
    qscale = (attn_scales / np.sqrt(np.float32(HEAD_DIM))).astype(np.float32)[None, :]

    # rotate-half permutation of q/k feature dims (per head), folded into
    # the projection weight columns: permuted feature j<64 <- 2j, j>=64 <- 2(j-64)+1
    perm = np.concatenate([np.arange(0, HEAD_DIM, 2), np.arange(1, HEAD_DIM, 2)])
    wq_p = wq.reshape(D, N_HEADS, HEAD_DIM)[:, :, perm].reshape(D, N_HEADS * HEAD_DIM)
    wk_p = wk[:, perm]

    def tile_x(xT):
        # [D, S] -> [sb, dg, p, c, s]
        return np.ascontiguousarray(
            xT.reshape(8, 4, P, NSB, SB).transpose(3, 0, 2, 1, 4)
        )

    def tile_w(w):
        # [D, m] -> [dg, p, c, m]
        m = w.shape[1]
        return np.ascontiguousarray(
            w.reshape(8, 4, P, m).transpose(0, 2, 1, 3)
        )

    def tile_wo(wg):
        # [256, D] -> [p, hh, cc, q]
        return np.ascontiguousarray(
            wg.reshape(HG, P, NCC, P).transpose(1, 0, 2, 3)
        )

    in_maps = []
    for core in range(8):
        b, g = core // 2, core % 2
        xT = np.ascontiguousarray(x[b].T).astype(npdt, copy=False)
        in_maps.append(
            {
                "xT": tile_x(xT),
                "wq_g": tile_w(
                    wq_p[:, g * HG * HEAD_DIM:(g + 1) * HG * HEAD_DIM].astype(npdt)
                ),
                "wk": tile_w(wk_p.astype(npdt)),
                "wv": tile_w(wv.astype(npdt)),
                "wo_g": tile_wo(
                    wo[g * HG * HEAD_DIM:(g + 1) * HG * HEAD_DIM, :].astype(npdt)
                ),
                "csT": csT,
                "snT": snT,
                "qscale": qscale,
            }
        )
    return in_maps


def kernel(x, positions, wq, wk, wv, wo, _trace=False, _trace_kwargs=None):
    x = np.asarray(x, np.float32)
    positions = np.asarray(positions)
    wq = np.asarray(wq, np.float32)
    wk = np.asarray(wk, np.float32)
    wv = np.asarray(wv, np.float32)
    wo = np.asarray(wo, np.float32)

    nc = build_bass()
    in_maps = _host_prep(x, positions, wq, wk, wv, wo)
    res = bass_utils.run_bass_kernel_spmd(
        nc, in_maps, core_ids=list(range(8)), trace=_trace,
        **(_trace_kwargs or {}),
    )
    kernel.last_results = res

    out = np.empty((B, S, D), np.float32)
    for b in range(B):
        pa = res.results[2 * b]["outT"].astype(np.float32)
        pb = res.results[2 * b + 1]["outT"].astype(np.float32)
        full = (pa + pb).transpose(0, 2, 1, 3).reshape(D, S)
        out[b] = full.T
    return out
